# revision 9
# baseline (speedup 1.0000x reference)
"""Trainium2 Bass kernel for nn_LowRankRotatedSpaceIntervention.

Reference computation (B=8192, D=4096, r=512, k=128):
    sel  = subspaces[0]                  # shared index set (fast path)
    diff = (source - base) @ W           # [B, r]
    out  = base + diff[:, sel] @ W[:, sel].T

Only the selected k=128 columns of W matter:
    out = base + ((source - base) @ W_sel) @ W_sel.T,  W_sel = W[:, sel]

Sharding: data-parallel over batch across 8 NeuronCores; W_sel (2 MiB)
replicated. Host precomputes W_sel and W_sel.T (cheap) from subspaces[0].

The kernel is HBM-bandwidth bound, so precision is cut wherever the
harness tolerance (2e-2) allows:
  - base/source land in SBUF as bf16 via SWDGE casting DMA (HBM reads
    stay f32; the cast frees SBUF so all 8 row-blocks stay resident)
  - both matmuls in bf16
  - output stored as fp16 (halves store traffic; host upcasts to f32)

Scheduling (per core: batch shard 1024 rows = 8 blocks of 128):
  - ALL loads are issued first on the SWDGE (gpsimd) queue, and the
    stores are issued on the SAME queue, so the SDMA rings serve every
    load before any store: the last block's inputs land at ~92us
    instead of being pushed behind interleaved stores.
  - front(i): per 1024-col group: sub (DVE), PE-transpose 8 chunks to
    psum, ACT copy to SBUF; mm1 lags the copies by one group; then
    T^T -> ttt (bf16).
  - back(i): per 512-col chunk: mm2 (bf16) to psum, DVE add with bb to
    fp16 ot; store halves (SWDGE) as they complete.
  - skew-1 software pipeline: front(i+1) issues before back(i) so the
    engine FIFOs never bury the next block's sub behind the previous
    block's adds.
"""

import os
import numpy as np
import ml_dtypes

import concourse.bass as bass
import concourse.tile as tile
from concourse import bacc, masks, mybir
from concourse.bass_utils import run_bass_kernel_spmd

N_CORES = 8
B_FULL = 8192
D = 4096
K = 128
BS = B_FULL // N_CORES  # 1024 rows per core
NB = BS // 128          # 8 blocks of 128 rows
NCH = D // 128          # 32 contraction chunks of 128

F32 = mybir.dt.float32
BF16 = mybir.dt.bfloat16
F16 = mybir.dt.float16

PER_BANK = 8            # bf16 [128,128] transposes per psum bank
GCOLS = 128 * PER_BANK  # 1024 columns per transpose group
NG = D // GCOLS         # 4 groups per block


def _build(db_bufs=2, dt_bufs=2, sb_halves=True, swdge_stores=False,
           gps_sub_groups=1):
    nc = bacc.Bacc("TRN2", target_bir_lowering=False, debug=False)

    base_d = nc.dram_tensor("base", [BS, D], F32, kind="ExternalInput").ap()
    src_d = nc.dram_tensor("source", [BS, D], F32, kind="ExternalInput").ap()
    # w1: chunk-major W_sel: w1[p, 128*j + k] = W_sel[128*j + p, k]
    w1_d = nc.dram_tensor("w1", [128, D], BF16, kind="ExternalInput").ap()
    # w2: W_sel.T  (k on partitions)
    w2_d = nc.dram_tensor("w2", [K, D], BF16, kind="ExternalInput").ap()
    out_d = nc.dram_tensor("out", [BS, D], F16, kind="ExternalOutput").ap()
    scr_d = nc.dram_tensor("scratch", [1, 16], F16, kind="Internal").ap()

    with tile.TileContext(nc) as tc:
        with (
            tc.tile_pool(name="wpool", bufs=1) as wpool,
            tc.tile_pool(name="ipool", bufs=1) as ipool,
            tc.tile_pool(name="bbpool", bufs=NB) as bbpool,
            tc.tile_pool(name="sbpool", bufs=NB) as sbpool,
            tc.tile_pool(name="dpool", bufs=db_bufs) as dpool,
            tc.tile_pool(name="dtpool", bufs=dt_bufs) as dtpool,
            tc.tile_pool(name="ttpool", bufs=2) as ttpool,
            tc.tile_pool(name="ptr", bufs=2, space="PSUM") as ptrpool,
            tc.tile_pool(name="pT", bufs=2, space="PSUM") as pTpool,
            tc.tile_pool(name="p2", bufs=4, space="PSUM") as p2pool,
        ):
            w1_sb = wpool.tile([128, D], BF16, tag="w1")
            nc.sync.dma_start(w1_sb[:], w1_d[:])
            w2_sb = wpool.tile([K, D], BF16, tag="w2")
            nc.sync.dma_start(w2_sb[:], w2_d[:])

            # all loads up front on the SWDGE queue, in block order;
            # base/source land as fp16 (cast in the SDMA datapath)
            bbs, sbs = [], []

            def issue_loads(i):
                rows = slice(128 * i, 128 * (i + 1))
                bb = bbpool.tile([128, D], F16, tag="bb")
                nc.gpsimd.dma_start(bb[:], base_d[rows, :])
                bbs.append(bb)
                sb = sbpool.tile([128, D], F16, tag="sb")
                if sb_halves:
                    h = D // 2
                    nc.gpsimd.dma_start(sb[:, :h], src_d[rows, :h])
                    nc.gpsimd.dma_start(sb[:, h:], src_d[rows, h:])
                else:
                    nc.gpsimd.dma_start(sb[:], src_d[rows, :])
                sbs.append(sb)

            issue_loads(0)
            if gps_sub_groups > 0:
                # pay the gpsimd tensor-op IRAM load while the first
                # block's loads drain, before the remaining issues
                warm = ipool.tile([128, 64], BF16, tag="warm")
                nc.gpsimd.memset(warm[:], 0.0)
                nc.gpsimd.tensor_sub(warm[:], warm[:], warm[:])
            for i in range(1, NB):
                issue_loads(i)

            ident = ipool.tile([128, 128], BF16, tag="ident")
            masks.make_identity(nc, ident[:])
            ident16 = ipool.tile([128, 128], F16, tag="ident16")
            masks.make_identity(nc, ident16[:])

            store_engine = nc.gpsimd if swdge_stores else nc.sync
            if not swdge_stores:
                # keep the HWDGE ring empty until the final load has
                # landed: loads keep full SDMA bandwidth, stores drain
                # at full rate afterwards (nothing upstream waits on
                # stores: the adds run in place and bb is never reused)
                nc.sync.dma_start(scr_d[:, :], sbs[-1][:1, D - 16 :])

            def front(i):
                bb, sb = bbs[i], sbs[i]
                db = dpool.tile([128, D], BF16, tag="db")
                dtt = dtpool.tile([128, D], BF16, tag="dtt")
                pt = pTpool.tile([K, 128], F32, tag="pt")

                def mm1_group(g):
                    for q in range(PER_BANK):
                        j = PER_BANK * g + q
                        nc.tensor.matmul(
                            pt[:],
                            w1_sb[:, 128 * j : 128 * (j + 1)],
                            dtt[:, 128 * j : 128 * (j + 1)],
                            start=(j == 0),
                            stop=(j == NCH - 1),
                        )

                for g in range(NG):
                    cols = slice(GCOLS * g, GCOLS * (g + 1))
                    sub_eng = nc.gpsimd if g < gps_sub_groups else nc.vector
                    sub_eng.tensor_sub(db[:, cols], sb[:, cols], bb[:, cols])
                    ps = ptrpool.tile([128, GCOLS], BF16, tag="ps")
                    for q in range(PER_BANK):
                        j = PER_BANK * g + q
                        nc.tensor.transpose(
                            ps[:, 128 * q : 128 * (q + 1)],
                            db[:, 128 * j : 128 * (j + 1)],
                            ident[:],
                        )
                    nc.scalar.copy(dtt[:, cols], ps[:])
                    # mm1 lags the copies by one group so the PE never
                    # stalls at the queue head waiting on the ACT copy
                    if g > 0:
                        mm1_group(g - 1)
                mm1_group(NG - 1)

                ttt = ttpool.tile([K, 128], BF16, tag="ttt")
                nc.scalar.copy(ttt[:], pt[:])
                return ttt

            def back(i, ttt):
                rows = slice(128 * i, 128 * (i + 1))
                bb = bbs[i]
                # the result lands in place in bb (its last reader), so
                # nothing upstream ever waits on store recycling.
                # chunks 0-3: DVE adds bb + psum; chunks 4-7: the PE
                # accumulates bb into the mm2 psum (identity matmul) and
                # the ACT engine writes it back — halves the DVE load
                # and runs the two output halves on different engines.
                for dj in range(D // 512):
                    cols = slice(512 * dj, 512 * (dj + 1))
                    dve_half = dj < D // 1024
                    p2t = p2pool.tile([128, 512], F32, tag="p2t")
                    nc.tensor.matmul(
                        p2t[:], ttt[:], w2_sb[:, cols],
                        start=True, stop=dve_half,
                    )
                    if dve_half:
                        nc.vector.tensor_add(bb[:, cols], bb[:, cols], p2t[:])
                    else:
                        nc.tensor.matmul(
                            p2t[:], ident16[:], bb[:, cols],
                            start=False, stop=True,
                        )
                        nc.scalar.copy(bb[:, cols], p2t[:])
                    if dj == D // 1024 - 1:
                        store_engine.dma_start(out_d[rows, : D // 2], bb[:, : D // 2])
                store_engine.dma_start(out_d[rows, D // 2 :], bb[:, D // 2 :])

            for k in range(NB):
                back(k, front(k))

    nc.compile()
    return nc


_NC_CACHE = {}


def _get_nc(**cfg):
    key = tuple(sorted(cfg.items()))
    if key not in _NC_CACHE:
        _NC_CACHE[key] = _build(**cfg)
    return _NC_CACHE[key]


def make_in_maps(inputs):
    base = np.ascontiguousarray(np.asarray(inputs["base"], dtype=np.float32))
    source = np.ascontiguousarray(np.asarray(inputs["source"], dtype=np.float32))
    subspaces = np.asarray(inputs["subspaces"])
    W = np.asarray(inputs["W"], dtype=np.float32)
    assert base.shape == (B_FULL, D) and source.shape == (B_FULL, D)

    sel = np.asarray(subspaces[0]).astype(np.int64)  # shared index set
    W_sel = np.ascontiguousarray(W[:, sel])          # [D, K] f32
    # chunk-major layout: w1[p, 128*j + k] = W_sel[128*j + p, k]
    w1 = np.ascontiguousarray(
        W_sel.reshape(NCH, 128, K).transpose(1, 0, 2).reshape(128, D)
    ).astype(ml_dtypes.bfloat16)
    w2 = np.ascontiguousarray(W_sel.T).astype(ml_dtypes.bfloat16)  # [K, D]

    in_maps = []
    for c in range(N_CORES):
        in_maps.append(
            {
                "base": np.ascontiguousarray(base[c * BS : (c + 1) * BS]),
                "source": np.ascontiguousarray(source[c * BS : (c + 1) * BS]),
                "w1": w1,
                "w2": w2,
            }
        )
    return in_maps


def run(inputs, trace=False, **cfg):
    nc = _get_nc(**cfg)
    in_maps = make_in_maps(inputs)
    res = run_bass_kernel_spmd(nc, in_maps, list(range(N_CORES)), trace=trace)
    out = np.concatenate(
        [np.asarray(r["out"], dtype=np.float32) for r in res.results], axis=0
    )
    return out, res


def _env_cfg():
    return dict(
        db_bufs=int(os.environ.get("LRI_DB_BUFS", "2")),
        dt_bufs=int(os.environ.get("LRI_DT_BUFS", "2")),
        sb_halves=os.environ.get("LRI_SB_HALVES", "1") == "1",
        swdge_stores=os.environ.get("LRI_SWDGE_STORES", "0") == "1",
        gps_sub_groups=int(os.environ.get("LRI_GPS_SUB_GROUPS", "0")),
    )


def kernel(**inputs) -> np.ndarray:
    out, _ = run(inputs, trace=False, **_env_cfg())
    return out


# revision 10
# speedup vs baseline: 1.1530x; 1.1530x over previous
"""Trainium2 Bass kernel for nn_LowRankRotatedSpaceIntervention.

Reference computation (B=8192, D=4096, r=512, k=128):
    sel  = subspaces[0]                  # shared index set (fast path)
    diff = (source - base) @ W           # [B, r]
    out  = base + diff[:, sel] @ W[:, sel].T

Only the selected k=128 columns of W matter:
    out = base + ((source - base) @ W_sel) @ W_sel.T,  W_sel = W[:, sel]

Sharding: data-parallel over batch across 8 NeuronCores; W_sel (2 MiB)
replicated. Host precomputes W_sel and W_sel.T (cheap) from subspaces[0].

The kernel is HBM-bandwidth bound, so precision is cut wherever the
harness tolerance (2e-2) allows:
  - base/source land in SBUF as bf16 via SWDGE casting DMA (HBM reads
    stay f32; the cast frees SBUF so all 8 row-blocks stay resident)
  - both matmuls in bf16
  - output stored as fp16 (halves store traffic; host upcasts to f32)

Scheduling (per core: batch shard 1024 rows = 8 blocks of 128):
  - ALL loads are issued first on the SWDGE (gpsimd) queue, and the
    stores are issued on the SAME queue, so the SDMA rings serve every
    load before any store: the last block's inputs land at ~92us
    instead of being pushed behind interleaved stores.
  - front(i): per 1024-col group: sub (DVE), PE-transpose 8 chunks to
    psum, ACT copy to SBUF; mm1 lags the copies by one group; then
    T^T -> ttt (bf16).
  - back(i): per 512-col chunk: mm2 (bf16) to psum, DVE add with bb to
    fp16 ot; store halves (SWDGE) as they complete.
  - skew-1 software pipeline: front(i+1) issues before back(i) so the
    engine FIFOs never bury the next block's sub behind the previous
    block's adds.
"""

import os
import numpy as np
import ml_dtypes

import concourse.bass as bass
import concourse.tile as tile
from concourse import bacc, masks, mybir
from concourse.bass_utils import run_bass_kernel_spmd

N_CORES = 8
B_FULL = 8192
D = 4096
K = 128
BS = B_FULL // N_CORES  # 1024 rows per core
NB = BS // 128          # 8 blocks of 128 rows
NCH = D // 128          # 32 contraction chunks of 128

F32 = mybir.dt.float32
BF16 = mybir.dt.bfloat16
F16 = mybir.dt.float16

PER_BANK = 8            # bf16 [128,128] transposes per psum bank
GCOLS = 128 * PER_BANK  # 1024 columns per transpose group
NG = D // GCOLS         # 4 groups per block


def _build(db_bufs=2, dt_bufs=2, sb_halves=True, swdge_stores=False,
           gps_sub_groups=1):
    nc = bacc.Bacc("TRN2", target_bir_lowering=False, debug=False)

    base_d = nc.dram_tensor("base", [BS, D], F32, kind="ExternalInput").ap()
    src_d = nc.dram_tensor("source", [BS, D], F32, kind="ExternalInput").ap()
    # w1: chunk-major W_sel: w1[p, 128*j + k] = W_sel[128*j + p, k]
    w1_d = nc.dram_tensor("w1", [128, D], BF16, kind="ExternalInput").ap()
    # w2: W_sel.T  (k on partitions)
    w2_d = nc.dram_tensor("w2", [K, D], BF16, kind="ExternalInput").ap()
    out_d = nc.dram_tensor("out", [BS, D], F16, kind="ExternalOutput").ap()
    scr_d = nc.dram_tensor("scratch", [1, 16], F16, kind="Internal").ap()

    with tile.TileContext(nc) as tc:
        with (
            tc.tile_pool(name="wpool", bufs=1) as wpool,
            tc.tile_pool(name="ipool", bufs=1) as ipool,
            tc.tile_pool(name="bbpool", bufs=NB) as bbpool,
            tc.tile_pool(name="sbpool", bufs=NB) as sbpool,
            tc.tile_pool(name="dpool", bufs=db_bufs) as dpool,
            tc.tile_pool(name="dtpool", bufs=dt_bufs) as dtpool,
            tc.tile_pool(name="ttpool", bufs=2) as ttpool,
            tc.tile_pool(name="ptr", bufs=2, space="PSUM") as ptrpool,
            tc.tile_pool(name="pT", bufs=2, space="PSUM") as pTpool,
            tc.tile_pool(name="p2", bufs=4, space="PSUM") as p2pool,
        ):
            w1_sb = wpool.tile([128, D], BF16, tag="w1")
            nc.sync.dma_start(w1_sb[:], w1_d[:])
            w2_sb = wpool.tile([K, D], BF16, tag="w2")
            nc.sync.dma_start(w2_sb[:], w2_d[:])

            # all loads up front on the SWDGE queue, in block order;
            # base/source land as fp16 (cast in the SDMA datapath)
            bbs, sbs = [], []

            def issue_loads(i):
                rows = slice(128 * i, 128 * (i + 1))
                bb = bbpool.tile([128, D], F16, tag="bb")
                nc.gpsimd.dma_start(bb[:], base_d[rows, :])
                bbs.append(bb)
                sb = sbpool.tile([128, D], F16, tag="sb")
                if sb_halves:
                    h = D // 2
                    nc.gpsimd.dma_start(sb[:, :h], src_d[rows, :h])
                    nc.gpsimd.dma_start(sb[:, h:], src_d[rows, h:])
                else:
                    nc.gpsimd.dma_start(sb[:], src_d[rows, :])
                sbs.append(sb)

            issue_loads(0)
            if gps_sub_groups > 0:
                # pay the gpsimd tensor-op IRAM load while the first
                # block's loads drain, before the remaining issues
                warm = ipool.tile([128, 64], BF16, tag="warm")
                nc.gpsimd.memset(warm[:], 0.0)
                nc.gpsimd.tensor_sub(warm[:], warm[:], warm[:])
            for i in range(1, NB):
                issue_loads(i)

            ident = ipool.tile([128, 128], BF16, tag="ident")
            masks.make_identity(nc, ident[:])
            ident16 = ipool.tile([128, 128], F16, tag="ident16")
            masks.make_identity(nc, ident16[:])

            store_engine = nc.gpsimd if swdge_stores else nc.sync
            if not swdge_stores:
                # keep the HWDGE ring empty until the final load has
                # landed: loads keep full SDMA bandwidth, stores drain
                # at full rate afterwards (nothing upstream waits on
                # stores: the adds run in place and bb is never reused)
                nc.sync.dma_start(scr_d[:, :], sbs[-1][:1, D - 16 :])

            def front(i):
                bb, sb = bbs[i], sbs[i]
                db = dpool.tile([128, D], BF16, tag="db")
                dtt = dtpool.tile([128, D], BF16, tag="dtt")
                pt = pTpool.tile([K, 128], F32, tag="pt")

                def mm1_group(g):
                    for q in range(PER_BANK):
                        j = PER_BANK * g + q
                        nc.tensor.matmul(
                            pt[:],
                            w1_sb[:, 128 * j : 128 * (j + 1)],
                            dtt[:, 128 * j : 128 * (j + 1)],
                            start=(j == 0),
                            stop=(j == NCH - 1),
                        )

                for g in range(NG):
                    cols = slice(GCOLS * g, GCOLS * (g + 1))
                    sub_eng = nc.gpsimd if g < gps_sub_groups else nc.vector
                    sub_eng.tensor_sub(db[:, cols], sb[:, cols], bb[:, cols])
                    ps = ptrpool.tile([128, GCOLS], BF16, tag="ps")
                    for q in range(PER_BANK):
                        j = PER_BANK * g + q
                        nc.tensor.transpose(
                            ps[:, 128 * q : 128 * (q + 1)],
                            db[:, 128 * j : 128 * (j + 1)],
                            ident[:],
                        )
                    nc.scalar.copy(dtt[:, cols], ps[:])
                    # mm1 lags the copies by one group so the PE never
                    # stalls at the queue head waiting on the ACT copy
                    if g > 0:
                        mm1_group(g - 1)
                mm1_group(NG - 1)

                ttt = ttpool.tile([K, 128], BF16, tag="ttt")
                nc.scalar.copy(ttt[:], pt[:])
                return ttt

            def back(i, ttt):
                rows = slice(128 * i, 128 * (i + 1))
                bb = bbs[i]
                # the result lands in place in bb (its last reader), so
                # nothing upstream ever waits on store recycling.
                # chunks 0-3: DVE adds bb + psum; chunks 4-7: the PE
                # accumulates bb into the mm2 psum (identity matmul) and
                # the ACT engine writes it back — halves the DVE load
                # and runs the two output halves on different engines.
                for dj in range(D // 512):
                    cols = slice(512 * dj, 512 * (dj + 1))
                    dve_half = dj < D // 1024
                    p2t = p2pool.tile([128, 512], F32, tag="p2t")
                    nc.tensor.matmul(
                        p2t[:], ttt[:], w2_sb[:, cols],
                        start=True, stop=dve_half,
                    )
                    if dve_half:
                        nc.vector.tensor_add(bb[:, cols], bb[:, cols], p2t[:])
                    else:
                        nc.tensor.matmul(
                            p2t[:], ident16[:], bb[:, cols],
                            start=False, stop=True,
                        )
                        nc.scalar.copy(bb[:, cols], p2t[:])
                    if dj == D // 1024 - 1:
                        store_engine.dma_start(out_d[rows, : D // 2], bb[:, : D // 2])
                store_engine.dma_start(out_d[rows, D // 2 :], bb[:, D // 2 :])

            # skew-1 software pipeline: front(k+1) issues before back(k)
            state = front(0)
            for k in range(NB):
                nxt = front(k + 1) if k + 1 < NB else None
                back(k, state)
                state = nxt

    nc.compile()
    return nc


_NC_CACHE = {}


def _get_nc(**cfg):
    key = tuple(sorted(cfg.items()))
    if key not in _NC_CACHE:
        _NC_CACHE[key] = _build(**cfg)
    return _NC_CACHE[key]


def make_in_maps(inputs):
    base = np.ascontiguousarray(np.asarray(inputs["base"], dtype=np.float32))
    source = np.ascontiguousarray(np.asarray(inputs["source"], dtype=np.float32))
    subspaces = np.asarray(inputs["subspaces"])
    W = np.asarray(inputs["W"], dtype=np.float32)
    assert base.shape == (B_FULL, D) and source.shape == (B_FULL, D)

    sel = np.asarray(subspaces[0]).astype(np.int64)  # shared index set
    W_sel = np.ascontiguousarray(W[:, sel])          # [D, K] f32
    # chunk-major layout: w1[p, 128*j + k] = W_sel[128*j + p, k]
    w1 = np.ascontiguousarray(
        W_sel.reshape(NCH, 128, K).transpose(1, 0, 2).reshape(128, D)
    ).astype(ml_dtypes.bfloat16)
    w2 = np.ascontiguousarray(W_sel.T).astype(ml_dtypes.bfloat16)  # [K, D]

    in_maps = []
    for c in range(N_CORES):
        in_maps.append(
            {
                "base": np.ascontiguousarray(base[c * BS : (c + 1) * BS]),
                "source": np.ascontiguousarray(source[c * BS : (c + 1) * BS]),
                "w1": w1,
                "w2": w2,
            }
        )
    return in_maps


def run(inputs, trace=False, **cfg):
    nc = _get_nc(**cfg)
    in_maps = make_in_maps(inputs)
    res = run_bass_kernel_spmd(nc, in_maps, list(range(N_CORES)), trace=trace)
    out = np.concatenate(
        [np.asarray(r["out"], dtype=np.float32) for r in res.results], axis=0
    )
    return out, res


def _env_cfg():
    return dict(
        db_bufs=int(os.environ.get("LRI_DB_BUFS", "2")),
        dt_bufs=int(os.environ.get("LRI_DT_BUFS", "2")),
        sb_halves=os.environ.get("LRI_SB_HALVES", "1") == "1",
        swdge_stores=os.environ.get("LRI_SWDGE_STORES", "0") == "1",
        gps_sub_groups=int(os.environ.get("LRI_GPS_SUB_GROUPS", "0")),
    )


def kernel(**inputs) -> np.ndarray:
    out, _ = run(inputs, trace=False, **_env_cfg())
    return out


# revision 11
# speedup vs baseline: 1.3966x; 1.2113x over previous
"""Trainium2 Bass kernel for nn_LowRankRotatedSpaceIntervention.

Reference computation (B=8192, D=4096, r=512, k=128):
    sel  = subspaces[0]                  # shared index set (fast path)
    diff = (source - base) @ W           # [B, r]
    out  = base + diff[:, sel] @ W[:, sel].T

Only the selected k=128 columns of W matter:
    out = base + ((source - base) @ W_sel) @ W_sel.T,  W_sel = W[:, sel]

Sharding: data-parallel over batch across 8 NeuronCores; W_sel (2 MiB)
replicated. Host precomputes W_sel and W_sel.T (cheap) from subspaces[0].

The kernel is HBM-bandwidth bound, so precision is cut wherever the
harness tolerance (2e-2) allows:
  - base/source land in SBUF as bf16 via SWDGE casting DMA (HBM reads
    stay f32; the cast frees SBUF so all 8 row-blocks stay resident)
  - both matmuls in bf16
  - output stored as fp16 (halves store traffic; host upcasts to f32)

Scheduling (per core: batch shard 1024 rows = 8 blocks of 128):
  - ALL loads are issued first on the SWDGE (gpsimd) queue, and the
    stores are issued on the SAME queue, so the SDMA rings serve every
    load before any store: the last block's inputs land at ~92us
    instead of being pushed behind interleaved stores.
  - front(i): per 1024-col group: sub (DVE), PE-transpose 8 chunks to
    psum, ACT copy to SBUF; mm1 lags the copies by one group; then
    T^T -> ttt (bf16).
  - back(i): per 512-col chunk: mm2 (bf16) to psum, DVE add with bb to
    fp16 ot; store halves (SWDGE) as they complete.
  - skew-1 software pipeline: front(i+1) issues before back(i) so the
    engine FIFOs never bury the next block's sub behind the previous
    block's adds.
"""

import os
import numpy as np
import ml_dtypes

import concourse.bass as bass
import concourse.tile as tile
from concourse import bacc, masks, mybir
from concourse.bass_utils import run_bass_kernel_spmd

N_CORES = 8
B_FULL = 8192
D = 4096
K = 128
BS = B_FULL // N_CORES  # 1024 rows per core
NB = BS // 128          # 8 blocks of 128 rows
NCH = D // 128          # 32 contraction chunks of 128

F32 = mybir.dt.float32
BF16 = mybir.dt.bfloat16
F16 = mybir.dt.float16

PER_BANK = 8            # bf16 [128,128] transposes per psum bank
GCOLS = 128 * PER_BANK  # 1024 columns per transpose group
NG = D // GCOLS         # 4 groups per block


def _build(db_bufs=2, dt_bufs=2, sb_halves=True, swdge_stores=False,
           gps_sub_groups=1):
    nc = bacc.Bacc("TRN2", target_bir_lowering=False, debug=False)

    base_d = nc.dram_tensor("base", [BS, D], F32, kind="ExternalInput").ap()
    src_d = nc.dram_tensor("source", [BS, D], F32, kind="ExternalInput").ap()
    # w1: chunk-major W_sel: w1[p, 128*j + k] = W_sel[128*j + p, k]
    w1_d = nc.dram_tensor("w1", [128, D], BF16, kind="ExternalInput").ap()
    # w2: W_sel.T  (k on partitions)
    w2_d = nc.dram_tensor("w2", [K, D], BF16, kind="ExternalInput").ap()
    out_d = nc.dram_tensor("out", [BS, D], F16, kind="ExternalOutput").ap()
    scr_d = nc.dram_tensor("scratch", [1, 16], F16, kind="Internal").ap()

    with tile.TileContext(nc) as tc:
        with (
            tc.tile_pool(name="wpool", bufs=1) as wpool,
            tc.tile_pool(name="ipool", bufs=1) as ipool,
            tc.tile_pool(name="bbpool", bufs=NB) as bbpool,
            tc.tile_pool(name="sbpool", bufs=NB) as sbpool,
            tc.tile_pool(name="dpool", bufs=db_bufs) as dpool,
            tc.tile_pool(name="dtpool", bufs=dt_bufs) as dtpool,
            tc.tile_pool(name="ttpool", bufs=2) as ttpool,
            tc.tile_pool(name="ptr", bufs=2, space="PSUM") as ptrpool,
            tc.tile_pool(name="pT", bufs=2, space="PSUM") as pTpool,
            tc.tile_pool(name="p2", bufs=4, space="PSUM") as p2pool,
        ):
            w1_sb = wpool.tile([128, D], BF16, tag="w1")
            nc.sync.dma_start(w1_sb[:], w1_d[:])
            w2_sb = wpool.tile([K, D], BF16, tag="w2")
            nc.sync.dma_start(w2_sb[:], w2_d[:])

            # identities FIRST: they are made on the gpsimd queue, and
            # the SWDGE load issues below backpressure on descriptor-ring
            # space — created after the loads they would block every
            # PE transpose until the ring drains (~70us)
            ident = ipool.tile([128, 128], BF16, tag="ident")
            masks.make_identity(nc, ident[:])
            ident16 = ipool.tile([128, 128], F16, tag="ident16")
            masks.make_identity(nc, ident16[:])

            # all loads up front on the SWDGE queue, in block order;
            # base/source land as fp16 (cast in the SDMA datapath)
            bbs, sbs = [], []

            def issue_loads(i):
                rows = slice(128 * i, 128 * (i + 1))
                bb = bbpool.tile([128, D], F16, tag="bb")
                nc.gpsimd.dma_start(bb[:], base_d[rows, :])
                bbs.append(bb)
                sb = sbpool.tile([128, D], F16, tag="sb")
                if sb_halves:
                    h = D // 2
                    nc.gpsimd.dma_start(sb[:, :h], src_d[rows, :h])
                    nc.gpsimd.dma_start(sb[:, h:], src_d[rows, h:])
                else:
                    nc.gpsimd.dma_start(sb[:], src_d[rows, :])
                sbs.append(sb)

            issue_loads(0)
            if gps_sub_groups > 0:
                # pay the gpsimd tensor-op IRAM load while the first
                # block's loads drain, before the remaining issues
                warm = ipool.tile([128, 64], BF16, tag="warm")
                nc.gpsimd.memset(warm[:], 0.0)
                nc.gpsimd.tensor_sub(warm[:], warm[:], warm[:])
            for i in range(1, NB):
                issue_loads(i)

            store_engine = nc.gpsimd if swdge_stores else nc.sync
            if not swdge_stores:
                # keep the HWDGE ring empty until the final load has
                # landed: loads keep full SDMA bandwidth, stores drain
                # at full rate afterwards (nothing upstream waits on
                # stores: the adds run in place and bb is never reused)
                nc.sync.dma_start(scr_d[:, :], sbs[-1][:1, D - 16 :])

            def front(i):
                bb, sb = bbs[i], sbs[i]
                db = dpool.tile([128, D], BF16, tag="db")
                dtt = dtpool.tile([128, D], BF16, tag="dtt")
                pt = pTpool.tile([K, 128], F32, tag="pt")

                def mm1_group(g):
                    for q in range(PER_BANK):
                        j = PER_BANK * g + q
                        nc.tensor.matmul(
                            pt[:],
                            w1_sb[:, 128 * j : 128 * (j + 1)],
                            dtt[:, 128 * j : 128 * (j + 1)],
                            start=(j == 0),
                            stop=(j == NCH - 1),
                        )

                for g in range(NG):
                    cols = slice(GCOLS * g, GCOLS * (g + 1))
                    sub_eng = nc.gpsimd if g < gps_sub_groups else nc.vector
                    sub_eng.tensor_sub(db[:, cols], sb[:, cols], bb[:, cols])
                    ps = ptrpool.tile([128, GCOLS], BF16, tag="ps")
                    for q in range(PER_BANK):
                        j = PER_BANK * g + q
                        nc.tensor.transpose(
                            ps[:, 128 * q : 128 * (q + 1)],
                            db[:, 128 * j : 128 * (j + 1)],
                            ident[:],
                        )
                    nc.scalar.copy(dtt[:, cols], ps[:])
                    # mm1 lags the copies by one group so the PE never
                    # stalls at the queue head waiting on the ACT copy
                    if g > 0:
                        mm1_group(g - 1)
                mm1_group(NG - 1)

                ttt = ttpool.tile([K, 128], BF16, tag="ttt")
                nc.scalar.copy(ttt[:], pt[:])
                return ttt

            def back(i, ttt):
                rows = slice(128 * i, 128 * (i + 1))
                bb = bbs[i]
                # the result lands in place in bb (its last reader), so
                # nothing upstream ever waits on store recycling.
                # chunks 0-3: DVE adds bb + psum; chunks 4-7: the PE
                # accumulates bb into the mm2 psum (identity matmul) and
                # the ACT engine writes it back — halves the DVE load
                # and runs the two output halves on different engines.
                for dj in range(D // 512):
                    cols = slice(512 * dj, 512 * (dj + 1))
                    dve_half = dj < D // 1024
                    p2t = p2pool.tile([128, 512], F32, tag="p2t")
                    nc.tensor.matmul(
                        p2t[:], ttt[:], w2_sb[:, cols],
                        start=True, stop=dve_half,
                    )
                    if dve_half:
                        nc.vector.tensor_add(bb[:, cols], bb[:, cols], p2t[:])
                    else:
                        nc.tensor.matmul(
                            p2t[:], ident16[:], bb[:, cols],
                            start=False, stop=True,
                        )
                        nc.scalar.copy(bb[:, cols], p2t[:])
                    if dj == D // 1024 - 1:
                        store_engine.dma_start(out_d[rows, : D // 2], bb[:, : D // 2])
                store_engine.dma_start(out_d[rows, D // 2 :], bb[:, D // 2 :])

            # skew-1 software pipeline: front(k+1) issues before back(k)
            state = front(0)
            for k in range(NB):
                nxt = front(k + 1) if k + 1 < NB else None
                back(k, state)
                state = nxt

    nc.compile()
    return nc


_NC_CACHE = {}


def _get_nc(**cfg):
    key = tuple(sorted(cfg.items()))
    if key not in _NC_CACHE:
        _NC_CACHE[key] = _build(**cfg)
    return _NC_CACHE[key]


def make_in_maps(inputs):
    base = np.ascontiguousarray(np.asarray(inputs["base"], dtype=np.float32))
    source = np.ascontiguousarray(np.asarray(inputs["source"], dtype=np.float32))
    subspaces = np.asarray(inputs["subspaces"])
    W = np.asarray(inputs["W"], dtype=np.float32)
    assert base.shape == (B_FULL, D) and source.shape == (B_FULL, D)

    sel = np.asarray(subspaces[0]).astype(np.int64)  # shared index set
    W_sel = np.ascontiguousarray(W[:, sel])          # [D, K] f32
    # chunk-major layout: w1[p, 128*j + k] = W_sel[128*j + p, k]
    w1 = np.ascontiguousarray(
        W_sel.reshape(NCH, 128, K).transpose(1, 0, 2).reshape(128, D)
    ).astype(ml_dtypes.bfloat16)
    w2 = np.ascontiguousarray(W_sel.T).astype(ml_dtypes.bfloat16)  # [K, D]

    in_maps = []
    for c in range(N_CORES):
        in_maps.append(
            {
                "base": np.ascontiguousarray(base[c * BS : (c + 1) * BS]),
                "source": np.ascontiguousarray(source[c * BS : (c + 1) * BS]),
                "w1": w1,
                "w2": w2,
            }
        )
    return in_maps


def run(inputs, trace=False, **cfg):
    nc = _get_nc(**cfg)
    in_maps = make_in_maps(inputs)
    res = run_bass_kernel_spmd(nc, in_maps, list(range(N_CORES)), trace=trace)
    out = np.concatenate(
        [np.asarray(r["out"], dtype=np.float32) for r in res.results], axis=0
    )
    return out, res


def _env_cfg():
    return dict(
        db_bufs=int(os.environ.get("LRI_DB_BUFS", "2")),
        dt_bufs=int(os.environ.get("LRI_DT_BUFS", "2")),
        sb_halves=os.environ.get("LRI_SB_HALVES", "1") == "1",
        swdge_stores=os.environ.get("LRI_SWDGE_STORES", "0") == "1",
        gps_sub_groups=int(os.environ.get("LRI_GPS_SUB_GROUPS", "0")),
    )


def kernel(**inputs) -> np.ndarray:
    out, _ = run(inputs, trace=False, **_env_cfg())
    return out
